# revision 17
# baseline (speedup 1.0000x reference)
"""Trainium2 Bass kernel: single-head causal attention.

B=4, T=4096, E=512, H=64, fp32 in/out.

Sharding: 2 cores per batch sample, split by keys (even/odd 128-strips via
a per-256-block half rotation baked in on the host). Each core computes a
partial softmax (numerator + denominator via a ones-column in V) over its
half of the keys for all 4096 queries; the host combines
out = (num0+num1)/(den0+den1).

v2 device kernel (per core):
  - Q projection uses [Wq|Wq] stationary so PSUM rows 0:64 and 64:128 both
    hold Q — the partition-64:128 copy feeds row-tiled score matmuls.
  - K^T is duplicated to partitions 64:128 of a side tile by a small
    SBUF->SBUF DMA on the gpsimd (SWDGE) queue.
  - Scores (contraction H=64) run as two concurrent row-tiled matmuls
    (tile_position (0,0) and (64,0)) per strip pair -> ~2x score rate.
  - PV is split into two 64-row matmuls (key halves) accumulating into two
    PSUM banks, combined during the DVE evacuation add. This keeps the
    whole attention stream in the 64-row PE tiling mode (no mode-switch
    drains between scores and PV).
  - V^T -> V natural transpose moved off the PE to the DMA xbar.
  - Diagonal trim: the 2nd diagonal strip is fully masked for query cols
    0:256 on both cores, so only 768 of 1024 columns are computed.
  - exp on the scalar engine with fused 1/sqrt(H) scale (no max subtract).
  - Warm-up matmuls at kernel start keep the PE HAM clock at 2.4 GHz.
  - Input x DMA is split across both HWDGE queues (scalar queue carries
    quarter 0) so the first projection can start early.
"""

import functools

import numpy as np
import ml_dtypes

B, T, E, H = 4, 4096, 512, 64
NCORES = 8
NCHUNK = 8  # 512-query chunks per sample
CHUNK = T // NCHUNK  # 512
NSTRIP = 16  # local 128-key strips per core (half of T/128)
VSTRIDE = 80  # per-strip stride in the packed V tile
NWARM = 16  # PE warm-up matmuls
PACKED_FROM = 2  # chunks >= this use row-packed scores (kdup ready by then)

bf16 = ml_dtypes.bfloat16


@functools.lru_cache(maxsize=1)
def _build():
    import concourse.mybir as mybir
    from concourse import bacc
    import concourse.tile as tile
    from concourse.masks import make_identity

    dt_bf = mybir.dt.bfloat16
    dt_f32 = mybir.dt.float32

    nc = bacc.Bacc("TRN2", target_bir_lowering=False, num_devices=NCORES)

    # x^T, rotated, (quarter, e-strip)-blocked: [4, 128, 4, 1024]
    xt = nc.dram_tensor("xt", [4, 128, 4, T // 4], dt_bf, kind="ExternalInput")
    wq2 = nc.dram_tensor("wq2", [128, 4 * 128], dt_bf, kind="ExternalInput")
    wkv = nc.dram_tensor("wkv", [128, 4 * 128], dt_bf, kind="ExternalInput")
    bias_q2 = nc.dram_tensor("bias_q2", [128, 1], dt_f32, kind="ExternalInput")
    bias_kv = nc.dram_tensor("bias_kv", [128, 1], dt_f32, kind="ExternalInput")
    masks = nc.dram_tensor("masks", [128, 768], dt_bf, kind="ExternalInput")
    # per chunk: [key-half-0 partial | key-half-1 partial], host adds them.
    # fp16 with a 2^-6 scale folded in (cancels in the host's num/den).
    dt_f16 = mybir.dt.float16
    out_d = nc.dram_tensor("out", [H + 1, 2 * T], dt_f16, kind="ExternalOutput")

    scale = 1.0 / float(np.sqrt(H))

    with tile.TileContext(nc) as tc:
        with (
            tc.tile_pool(name="const", bufs=1) as cpool,
            tc.tile_pool(name="xt_pool", bufs=1) as xpool,
            tc.tile_pool(name="q_pool", bufs=3) as qpool,
            tc.tile_pool(name="kv_pool", bufs=4) as kvpool,
            tc.tile_pool(name="kd_pool", bufs=4) as kdpool,
            tc.tile_pool(name="v_pool", bufs=1) as vpool,
            tc.tile_pool(name="p_pool", bufs=4) as ppool,
            tc.tile_pool(name="o_pool", bufs=2) as opool,
            tc.tile_pool(name="ps_proj", bufs=2, space="PSUM") as pspr,
            tc.tile_pool(name="ps_s", bufs=2, space="PSUM") as pss,
            tc.tile_pool(name="ps_o", bufs=1, space="PSUM") as pso,
        ):
            # ---- input DMAs: x quarter 0 strictly first on the sync
            # queue (DMA bandwidth is shared across queues, so the
            # critical first quarter must not compete); tiny weight
            # tensors ride the scalar HWDGE queue ----
            xt_sb = xpool.tile([128, 4 * T], dt_bf)

            def xt_block(qd, es):
                off = (qd * 4 + es) * 1024
                return xt_sb[:, off : off + 1024]

            def xt_dma(eng, qd, half):  # half 0 -> es 0,1 ; 1 -> es 2,3
                eng.dma_start(
                    xt_sb[
                        :, (qd * 4 + 2 * half) * 1024 : (qd * 4 + 2 * half + 2) * 1024
                    ],
                    xt.ap()[qd][:, 2 * half : 2 * half + 2, :].rearrange(
                        "p a t -> p (a t)"
                    ),
                )

            wkv_sb = cpool.tile([128, 512], dt_bf)
            nc.sync.dma_start(wkv_sb, wkv.ap())
            xt_dma(nc.sync, 0, 0)
            xt_dma(nc.sync, 0, 1)
            wq2_sb = cpool.tile([128, 512], dt_bf)
            nc.sync.dma_start(wq2_sb, wq2.ap())
            bkv_sb = cpool.tile([128, 1], dt_f32)
            nc.sync.dma_start(bkv_sb, bias_kv.ap())
            bq2_sb = cpool.tile([128, 1], dt_f32)
            nc.sync.dma_start(bq2_sb, bias_q2.ap())
            masks_sb = cpool.tile([128, 768], dt_bf)
            nc.sync.dma_start(masks_sb, masks.ap())
            xt_dma(nc.sync, 1, 0)
            xt_dma(nc.sync, 1, 1)
            xt_dma(nc.sync, 2, 0)
            xt_dma(nc.sync, 2, 1)
            xt_dma(nc.sync, 3, 0)
            xt_dma(nc.sync, 3, 1)

            # ---- PE warm-up: back-to-back junk matmuls flip the HAM
            # clock gate to 2.4 GHz while the first x DMAs land ----
            zt = cpool.tile([128, 128], dt_bf)
            nc.vector.memset(zt, 0.0)
            ident = cpool.tile([128, 128], dt_bf)
            make_identity(nc, ident)
            ps_w = pspr.tile([128, 512], dt_f32, tag="proj")
            for _ in range(NWARM):
                nc.tensor.matmul(ps_w[:, 0:128], lhsT=zt, rhs=zt, start=True, stop=True)

            # packed V (natural [k,h] layout + ones column for denominator)
            v_nat = vpool.tile([128, NSTRIP * VSTRIDE], dt_bf)
            v3 = v_nat.rearrange("p (s c) -> p s c", c=VSTRIDE)
            nc.vector.memset(v3[:, :, 64:65], 1.0)

            kv_tiles = []
            kd_tiles = []
            q_tiles = []

            def kv_proj(ckv):
                ps_kv = pspr.tile([128, 512], dt_f32, tag="proj")
                for es in range(4):
                    # keys: first 128 tokens of each 256-block
                    key_rhs = xt_block(ckv, es).rearrange(
                        "p (a two b) -> p a two b", two=2, b=128
                    )[:, :, 0, :]
                    nc.tensor.matmul(
                        ps_kv,
                        lhsT=wkv_sb[:, es * 128 : (es + 1) * 128],
                        rhs=key_rhs,
                        start=(es == 0),
                        stop=(es == 3),
                    )
                kv_sb = kvpool.tile([128, 512], dt_bf, tag="kv")
                nc.vector.tensor_scalar_add(kv_sb, ps_kv, bkv_sb)
                kv_tiles.append(kv_sb)
                # K^T duplicate at partitions 64:128 for row-tiled scores
                kd = kdpool.tile([128, 512], dt_bf, tag="kd")
                nc.gpsimd.dma_start(kd[64:128, :], kv_sb[0:64, :])
                kd_tiles.append(kd)

            def vtrans_pe(ckv, js):
                # V^T blocks -> natural V strips via PE transpose (early
                # chunks: needed within ~1us of the kv projection)
                kv_sb = kv_tiles[ckv]
                for j in js:
                    s = 4 * ckv + j
                    ps_tr = pspr.tile([128, 128], dt_bf, tag="proj")
                    nc.tensor.transpose(
                        ps_tr, kv_sb[:, j * 128 : (j + 1) * 128], ident
                    )
                    nc.vector.tensor_copy(
                        v_nat[:, s * VSTRIDE : s * VSTRIDE + 64],
                        ps_tr[:, 64:128],
                    )

            def vtrans_dma(ckv):
                # late kv chunks: DMA xbar transpose (sync queue slack)
                kv_sb = kv_tiles[ckv]
                for j in range(4):
                    s = 4 * ckv + j
                    nc.sync.dma_start(
                        v_nat[:, s * VSTRIDE : s * VSTRIDE + 64],
                        kv_sb[64:128, j * 128 : (j + 1) * 128],
                        transpose=True,
                    )

            def q_proj(c):
                ps_q = pspr.tile([128, 512], dt_f32, tag="proj")
                for es in range(4):
                    nc.tensor.matmul(
                        ps_q,
                        lhsT=wq2_sb[:, es * 128 : (es + 1) * 128],
                        rhs=xt_block(c // 2, es)[
                            :, (c % 2) * CHUNK : (c % 2) * CHUNK + CHUNK
                        ],
                        start=(es == 0),
                        stop=(es == 3),
                    )
                q_sb = qpool.tile([128, 512], dt_bf, tag="q")
                nc.vector.tensor_scalar_add(q_sb, ps_q, bq2_sb)
                q_tiles.append(q_sb)

            def emit_S(c, g):
                """Scores for strip pair g of chunk c: strip 2g (512 query
                cols) and strip 2g+1 (256 cols if diagonal, else 512)."""
                diag = g == c
                w2 = 256 if diag else 512
                ps = pss.tile([128, 1024], dt_f32, tag="pss")
                q = q_tiles[c]
                l0, l1 = 2 * g, 2 * g + 1
                lt0 = kv_tiles[l0 // 4][0:64, (l0 % 4) * 128 : (l0 % 4 + 1) * 128]
                if c >= PACKED_FROM:
                    # concurrent row-tiled pair: (0,0) and (64,0)
                    lt1 = kd_tiles[l1 // 4][64:128, (l1 % 4) * 128 : (l1 % 4 + 1) * 128]
                    r1 = q[64:128, 512 - w2 : 512]
                else:
                    lt1 = kv_tiles[l1 // 4][0:64, (l1 % 4) * 128 : (l1 % 4 + 1) * 128]
                    r1 = q[0:64, 512 - w2 : 512]
                nc.tensor.matmul(
                    ps[:, 0:512], lhsT=lt0, rhs=q[0:64, :], start=True, stop=True
                )
                nc.tensor.matmul(
                    ps[:, 512 : 512 + w2], lhsT=lt1, rhs=r1, start=True, stop=True
                )
                return ps

            def emit_E(c, g, ps):
                diag = g == c
                w = 768 if diag else 1024
                p = ppool.tile([128, 1024], dt_bf, tag="p")
                nc.scalar.activation(
                    p[:, 0:w],
                    ps[:, 0:w],
                    mybir.ActivationFunctionType.Exp,
                    scale=scale,
                )
                if diag:
                    nc.vector.tensor_mul(p[:, 0:768], p[:, 0:768], masks_sb)
                return p

            def emit_V(c, g, p, pso_t, first, last):
                """PV for strip pair g, split into key halves h0/h1 (two
                concurrent 64-row matmuls into separate PSUM banks).
                first/last flag the chunk's accumulation group bounds."""
                diag = g == c
                w2 = 256 if diag else 512
                for i, (l, pc0, pc1, oc0) in enumerate(
                    (
                        (2 * g, 0, 512, 0),
                        (2 * g + 1, 512, 512 + w2, 512 - w2),
                    )
                ):
                    start = first and i == 0
                    stop = last and i == 1
                    vs = v_nat[:, l * VSTRIDE : l * VSTRIDE + 65]
                    nc.tensor.matmul(
                        pso_t[:, oc0:512],
                        lhsT=vs[0:64, :],
                        rhs=p[0:64, pc0:pc1],
                        start=start,
                        stop=stop,
                    )
                    nc.tensor.matmul(
                        pso_t[:, 512 + oc0 : 1024],
                        lhsT=vs[64:128, :],
                        rhs=p[64:128, pc0:pc1],
                        start=start,
                        stop=stop,
                    )

            def emit_O(c, pso_t):
                # single-PSUM-input copy (DVE has one PSUM read port); the
                # host adds the two key-half partials
                o = opool.tile([H + 1, 1024], dt_f16, tag="o")
                nc.vector.tensor_scalar_mul(o, pso_t, 2.0**-6)
                nc.sync.dma_start(out_d.ap()[:, c * 1024 : (c + 1) * 1024], o)

            def proj_filler(c):
                # emitted after the diag PV of chunk c; fills PE idle slots
                if c == 0:
                    vtrans_pe(0, (2, 3))
                    kv_proj(1)
                    q_proj(2)
                elif c == 1:
                    q_proj(3)
                    vtrans_pe(1, (0, 1, 2, 3))
                elif c == 2:
                    q_proj(4)
                    kv_proj(2)
                    vtrans_dma(2)
                elif c == 3:
                    q_proj(5)
                elif c == 4:
                    q_proj(6)
                    kv_proj(3)
                    vtrans_dma(3)
                elif c == 5:
                    q_proj(7)

            # ---- software-pipelined main loop: scores run two pairs
            # ahead of exp; PV trails exp by one pair. The DIAGONAL pair
            # goes FIRST in each chunk so the masked PV (the only one
            # gated on the vector engine) is off the chunk-tail chain ----
            all_pairs = [
                (c, g)
                for c in range(NCHUNK)
                for g in ([c] + list(range(c)))
            ]
            ps_map = {}
            sptr = 0

            def pump_S(n):
                nonlocal sptr
                for _ in range(n):
                    if sptr < len(all_pairs):
                        cc, gg = all_pairs[sptr]
                        ps_map[(cc, gg)] = emit_S(cc, gg)
                        sptr += 1

            kv_proj(0)
            q_proj(0)
            q_proj(1)
            pump_S(2)
            pso_t = None
            for i, (c, g) in enumerate(all_pairs):
                first = g == c  # diag pair leads the chunk
                last = (g == c - 1) or (c == 0)
                if first:
                    pso_t = pso.tile([H + 1, 1024], dt_f32, tag="pso")
                p = emit_E(c, g, ps_map.pop((c, g)))
                pump_S(1)
                if i == 0:
                    vtrans_pe(0, (0, 1))  # V strips for the first PV
                emit_V(c, g, p, pso_t, first, last)
                if first:
                    proj_filler(c)
                if last:
                    emit_O(c, pso_t)

    nc.compile()
    return nc


def _perm(rho):
    """Rotated-order permutation: rotated position i holds original token
    perm[i]. Involutive (half swap within each 256-block)."""
    i = np.arange(T)
    return (i // 256) * 256 + ((i % 256) + 128 * rho) % 256


def _make_in_maps(x, Wq, bq, Wk, bk, Wv, bv):
    # [Wq|Wq] per e-strip: the partition-64:128 copy of Q feeds the
    # row-tiled score matmuls.
    wq4 = Wq.reshape(4, 128, 64)
    wq2_pack = np.ascontiguousarray(
        np.concatenate([wq4, wq4], axis=2).transpose(1, 0, 2).reshape(128, 512)
    ).astype(bf16)
    wkv_pack = np.ascontiguousarray(
        np.concatenate([Wk.reshape(4, 128, 64), Wv.reshape(4, 128, 64)], axis=2)
        .transpose(1, 0, 2)
        .reshape(128, 512)
    ).astype(bf16)
    bias_q2 = np.ascontiguousarray(
        np.concatenate([bq, bq])[:, None]
    ).astype(np.float32)
    bias_kv = np.ascontiguousarray(np.concatenate([bk, bv])[:, None]).astype(
        np.float32
    )

    kk = np.arange(128)[:, None]
    in_maps = []
    for b in range(B):
        xt_b = np.ascontiguousarray(x[b].T).astype(bf16).reshape(4, 128, T)
        for rho in range(2):
            perm = _perm(rho)
            xt_rot = xt_b[:, :, perm]  # rotated token order
            xt_in = np.ascontiguousarray(
                xt_rot.reshape(4, 128, 4, T // 4).transpose(2, 1, 0, 3)
            )
            # masks: columns in rotated order; v = original within-chunk
            # offset of rotated column j (chunk-independent). m1 is zero
            # on query cols 0:256 for both cores -> only cols 256:512 kept.
            v = perm[:CHUNK]
            m0 = (kk - v[None, :] <= -128 * rho).astype(bf16)
            m1 = (kk - v[None, :] <= -256 - 128 * rho).astype(bf16)
            masks_np = np.ascontiguousarray(
                np.concatenate([m0, m1[:, 256:512]], axis=1)
            )
            in_maps.append(
                {
                    "xt": xt_in,
                    "wq2": wq2_pack,
                    "wkv": wkv_pack,
                    "bias_q2": bias_q2,
                    "bias_kv": bias_kv,
                    "masks": masks_np,
                }
            )
    return in_maps


def _combine(results):
    out = np.empty((B, T, H), np.float32)
    p1 = _perm(1)
    for b in range(B):
        # fold the two key-half partials: [65, 8, 2, 512] -> [65, 4096]
        a0 = (
            results[2 * b]["out"]
            .astype(np.float64)
            .reshape(H + 1, NCHUNK, 2, CHUNK)
            .sum(axis=2)
            .reshape(H + 1, T)
        )
        a1 = (
            results[2 * b + 1]["out"]
            .astype(np.float64)
            .reshape(H + 1, NCHUNK, 2, CHUNK)
            .sum(axis=2)
            .reshape(H + 1, T)
        )
        a1 = a1[:, p1]  # un-rotate core-1 columns (involutive perm)
        num = a0[:H] + a1[:H]
        den = a0[H] + a1[H]
        out[b] = (num / den).T.astype(np.float32)
    return out


def _run(trace=False, **inputs):
    from concourse import bass_utils

    nc = _build()
    in_maps = _make_in_maps(
        np.asarray(inputs["x"], np.float32),
        np.asarray(inputs["Wq"], np.float32),
        np.asarray(inputs["bq"], np.float32),
        np.asarray(inputs["Wk"], np.float32),
        np.asarray(inputs["bk"], np.float32),
        np.asarray(inputs["Wv"], np.float32),
        np.asarray(inputs["bv"], np.float32),
    )
    res = bass_utils.run_bass_kernel_spmd(
        nc, in_maps, list(range(NCORES)), trace=trace
    )
    return _combine(res.results), res.exec_time_ns


def kernel(**inputs):
    out, _ = _run(trace=False, **inputs)
    return out


# revision 18
# speedup vs baseline: 1.1335x; 1.1335x over previous
"""Trainium2 Bass kernel: single-head causal attention.

B=4, T=4096, E=512, H=64, fp32 in/out.

Sharding: 2 cores per batch sample, split by keys (even/odd 128-strips via
a per-256-block half rotation baked in on the host). Each core computes a
partial softmax (numerator + denominator via a ones-column in V) over its
half of the keys for all 4096 queries; the host combines
out = (num0+num1)/(den0+den1).

v2 device kernel (per core):
  - Q projection uses [Wq|Wq] stationary so PSUM rows 0:64 and 64:128 both
    hold Q — the partition-64:128 copy feeds row-tiled score matmuls.
  - K^T is duplicated to partitions 64:128 of a side tile by a small
    SBUF->SBUF DMA on the gpsimd (SWDGE) queue.
  - Scores (contraction H=64) run as two concurrent row-tiled matmuls
    (tile_position (0,0) and (64,0)) per strip pair -> ~2x score rate.
  - PV is split into two 64-row matmuls (key halves) accumulating into two
    PSUM banks, combined during the DVE evacuation add. This keeps the
    whole attention stream in the 64-row PE tiling mode (no mode-switch
    drains between scores and PV).
  - V^T -> V natural transpose moved off the PE to the DMA xbar.
  - Diagonal trim: the 2nd diagonal strip is fully masked for query cols
    0:256 on both cores, so only 768 of 1024 columns are computed.
  - exp on the scalar engine with fused 1/sqrt(H) scale (no max subtract).
  - Warm-up matmuls at kernel start keep the PE HAM clock at 2.4 GHz.
  - Input x DMA is split across both HWDGE queues (scalar queue carries
    quarter 0) so the first projection can start early.
"""

import functools

import numpy as np
import ml_dtypes

B, T, E, H = 4, 4096, 512, 64
NCORES = 8
NCHUNK = 8  # 512-query chunks per sample
CHUNK = T // NCHUNK  # 512
NSTRIP = 16  # local 128-key strips per core (half of T/128)
VSTRIDE = 80  # per-strip stride in the packed V tile
NWARM = 16  # PE warm-up matmuls
PACKED_FROM = 2  # chunks >= this use row-packed scores (kdup ready by then)
# strip pairs whose exp runs on the DVE via the Schraudolph bit-trick
# (softmax normalization cancels most of the ~3% element error; verified
# 2.7e-3 worst-element impact on the real data)
SCH_OFFLOAD = {(4, 1), (4, 3), (5, 1), (5, 3), (6, 1), (6, 3), (7, 1), (7, 3)}
SCH_MUL = 12102203.16 / 8.0  # log2(e)*2^23 * softmax scale (1/sqrt(64))
SCH_ADD = float(127 * 2**23 - 366500)  # bias calibrated vs np.exp

bf16 = ml_dtypes.bfloat16


@functools.lru_cache(maxsize=1)
def _build():
    import concourse.mybir as mybir
    from concourse import bacc
    import concourse.tile as tile
    from concourse.masks import make_identity

    dt_bf = mybir.dt.bfloat16
    dt_f32 = mybir.dt.float32

    nc = bacc.Bacc("TRN2", target_bir_lowering=False, num_devices=NCORES)

    # x^T, rotated, (quarter, e-strip)-blocked: [4, 128, 4, 1024]
    xt = nc.dram_tensor("xt", [4, 128, 4, T // 4], dt_bf, kind="ExternalInput")
    wq2 = nc.dram_tensor("wq2", [128, 4 * 128], dt_bf, kind="ExternalInput")
    wkv = nc.dram_tensor("wkv", [128, 4 * 128], dt_bf, kind="ExternalInput")
    bias_q2 = nc.dram_tensor("bias_q2", [128, 1], dt_f32, kind="ExternalInput")
    bias_kv = nc.dram_tensor("bias_kv", [128, 1], dt_f32, kind="ExternalInput")
    masks = nc.dram_tensor("masks", [128, 768], dt_bf, kind="ExternalInput")
    # per chunk: [key-half-0 partial | key-half-1 partial], host adds them.
    # fp16 with a 2^-6 scale folded in (cancels in the host's num/den).
    dt_f16 = mybir.dt.float16
    out_d = nc.dram_tensor("out", [H + 1, 2 * T], dt_f16, kind="ExternalOutput")

    scale = 1.0 / float(np.sqrt(H))

    with tile.TileContext(nc) as tc:
        with (
            tc.tile_pool(name="const", bufs=1) as cpool,
            tc.tile_pool(name="xt_pool", bufs=1) as xpool,
            tc.tile_pool(name="q_pool", bufs=3) as qpool,
            tc.tile_pool(name="kv_pool", bufs=4) as kvpool,
            tc.tile_pool(name="kd_pool", bufs=4) as kdpool,
            tc.tile_pool(name="v_pool", bufs=1) as vpool,
            tc.tile_pool(name="p_pool", bufs=4) as ppool,
            tc.tile_pool(name="o_pool", bufs=2) as opool,
            tc.tile_pool(name="i_pool", bufs=2) as ipool,
            tc.tile_pool(name="ps_proj", bufs=2, space="PSUM") as pspr,
            tc.tile_pool(name="ps_s", bufs=2, space="PSUM") as pss,
            tc.tile_pool(name="ps_o", bufs=1, space="PSUM") as pso,
        ):
            # ---- input DMAs: x quarter 0 strictly first on the sync
            # queue (DMA bandwidth is shared across queues, so the
            # critical first quarter must not compete); tiny weight
            # tensors ride the scalar HWDGE queue ----
            xt_sb = xpool.tile([128, 4 * T], dt_bf)

            def xt_block(qd, es):
                off = (qd * 4 + es) * 1024
                return xt_sb[:, off : off + 1024]

            def xt_dma(eng, qd, half):  # half 0 -> es 0,1 ; 1 -> es 2,3
                eng.dma_start(
                    xt_sb[
                        :, (qd * 4 + 2 * half) * 1024 : (qd * 4 + 2 * half + 2) * 1024
                    ],
                    xt.ap()[qd][:, 2 * half : 2 * half + 2, :].rearrange(
                        "p a t -> p (a t)"
                    ),
                )

            wkv_sb = cpool.tile([128, 512], dt_bf)
            nc.sync.dma_start(wkv_sb, wkv.ap())
            xt_dma(nc.sync, 0, 0)
            xt_dma(nc.sync, 0, 1)
            wq2_sb = cpool.tile([128, 512], dt_bf)
            nc.sync.dma_start(wq2_sb, wq2.ap())
            bkv_sb = cpool.tile([128, 1], dt_f32)
            nc.sync.dma_start(bkv_sb, bias_kv.ap())
            bq2_sb = cpool.tile([128, 1], dt_f32)
            nc.sync.dma_start(bq2_sb, bias_q2.ap())
            masks_sb = cpool.tile([128, 768], dt_bf)
            nc.sync.dma_start(masks_sb, masks.ap())
            xt_dma(nc.sync, 1, 0)
            xt_dma(nc.sync, 1, 1)
            xt_dma(nc.sync, 2, 0)
            xt_dma(nc.sync, 2, 1)
            xt_dma(nc.sync, 3, 0)
            xt_dma(nc.sync, 3, 1)

            # ---- PE warm-up: back-to-back junk matmuls flip the HAM
            # clock gate to 2.4 GHz while the first x DMAs land ----
            zt = cpool.tile([128, 128], dt_bf)
            nc.vector.memset(zt, 0.0)
            ident = cpool.tile([128, 128], dt_bf)
            make_identity(nc, ident)
            ps_w = pspr.tile([128, 512], dt_f32, tag="proj")
            for _ in range(NWARM):
                nc.tensor.matmul(ps_w[:, 0:128], lhsT=zt, rhs=zt, start=True, stop=True)

            # packed V (natural [k,h] layout + ones column for denominator)
            v_nat = vpool.tile([128, NSTRIP * VSTRIDE], dt_bf)
            v3 = v_nat.rearrange("p (s c) -> p s c", c=VSTRIDE)
            nc.vector.memset(v3[:, :, 64:65], 1.0)

            kv_tiles = []
            kd_tiles = []
            q_tiles = []

            def kv_proj(ckv):
                ps_kv = pspr.tile([128, 512], dt_f32, tag="proj")
                for es in range(4):
                    # keys: first 128 tokens of each 256-block
                    key_rhs = xt_block(ckv, es).rearrange(
                        "p (a two b) -> p a two b", two=2, b=128
                    )[:, :, 0, :]
                    nc.tensor.matmul(
                        ps_kv,
                        lhsT=wkv_sb[:, es * 128 : (es + 1) * 128],
                        rhs=key_rhs,
                        start=(es == 0),
                        stop=(es == 3),
                    )
                kv_sb = kvpool.tile([128, 512], dt_bf, tag="kv")
                nc.vector.tensor_scalar_add(kv_sb, ps_kv, bkv_sb)
                kv_tiles.append(kv_sb)
                # K^T duplicate at partitions 64:128 for row-tiled scores
                kd = kdpool.tile([128, 512], dt_bf, tag="kd")
                nc.gpsimd.dma_start(kd[64:128, :], kv_sb[0:64, :])
                kd_tiles.append(kd)
                # V^T blocks -> natural V strips. Early kv chunks go via
                # the PE (needed within ~1us, batched here to minimize
                # PE tiling-mode switches); late ones via the DMA xbar.
                for j in range(4):
                    s = 4 * ckv + j
                    if ckv < 2:
                        ps_tr = pspr.tile([128, 128], dt_bf, tag="proj")
                        nc.tensor.transpose(
                            ps_tr, kv_sb[:, j * 128 : (j + 1) * 128], ident
                        )
                        nc.vector.tensor_copy(
                            v_nat[:, s * VSTRIDE : s * VSTRIDE + 64],
                            ps_tr[:, 64:128],
                        )
                    else:
                        nc.sync.dma_start(
                            v_nat[:, s * VSTRIDE : s * VSTRIDE + 64],
                            kv_sb[64:128, j * 128 : (j + 1) * 128],
                            transpose=True,
                        )

            def q_proj(c):
                ps_q = pspr.tile([128, 512], dt_f32, tag="proj")
                for es in range(4):
                    nc.tensor.matmul(
                        ps_q,
                        lhsT=wq2_sb[:, es * 128 : (es + 1) * 128],
                        rhs=xt_block(c // 2, es)[
                            :, (c % 2) * CHUNK : (c % 2) * CHUNK + CHUNK
                        ],
                        start=(es == 0),
                        stop=(es == 3),
                    )
                q_sb = qpool.tile([128, 512], dt_bf, tag="q")
                nc.vector.tensor_scalar_add(q_sb, ps_q, bq2_sb)
                q_tiles.append(q_sb)

            def emit_S(c, g):
                """Scores for strip pair g of chunk c: strip 2g (512 query
                cols) and strip 2g+1 (256 cols if diagonal, else 512)."""
                diag = g == c
                w2 = 256 if diag else 512
                ps = pss.tile([128, 1024], dt_f32, tag="pss")
                q = q_tiles[c]
                l0, l1 = 2 * g, 2 * g + 1
                lt0 = kv_tiles[l0 // 4][0:64, (l0 % 4) * 128 : (l0 % 4 + 1) * 128]
                if c >= PACKED_FROM:
                    # concurrent row-tiled pair: (0,0) and (64,0)
                    lt1 = kd_tiles[l1 // 4][64:128, (l1 % 4) * 128 : (l1 % 4 + 1) * 128]
                    r1 = q[64:128, 512 - w2 : 512]
                else:
                    lt1 = kv_tiles[l1 // 4][0:64, (l1 % 4) * 128 : (l1 % 4 + 1) * 128]
                    r1 = q[0:64, 512 - w2 : 512]
                nc.tensor.matmul(
                    ps[:, 0:512], lhsT=lt0, rhs=q[0:64, :], start=True, stop=True
                )
                nc.tensor.matmul(
                    ps[:, 512 : 512 + w2], lhsT=lt1, rhs=r1, start=True, stop=True
                )
                return ps

            def emit_E(c, g, ps):
                diag = g == c
                w = 768 if diag else 1024
                p = ppool.tile([128, 1024], dt_bf, tag="p")
                if (c, g) in SCH_OFFLOAD:
                    # exp on the DVE: i = int(s*log2(e)*2^23/8 + C), then
                    # reinterpret the int32 bits as fp32 (~3% max rel err,
                    # cancels in the softmax normalization)
                    ib = ipool.tile([128, 1024], mybir.dt.int32, tag="ib")
                    nc.vector.tensor_scalar(
                        ib,
                        ps[:, 0:1024],
                        SCH_MUL,
                        SCH_ADD,
                        mybir.AluOpType.mult,
                        mybir.AluOpType.add,
                    )
                    nc.vector.tensor_copy(p, ib[:, :].bitcast(dt_f32))
                    return p
                nc.scalar.activation(
                    p[:, 0:w],
                    ps[:, 0:w],
                    mybir.ActivationFunctionType.Exp,
                    scale=scale,
                )
                if diag:
                    nc.vector.tensor_mul(p[:, 0:768], p[:, 0:768], masks_sb)
                return p

            def emit_V(c, g, p, pso_t, first, last):
                """PV for strip pair g, split into key halves h0/h1 (two
                concurrent 64-row matmuls into separate PSUM banks).
                first/last flag the chunk's accumulation group bounds."""
                diag = g == c
                w2 = 256 if diag else 512
                for i, (l, pc0, pc1, oc0) in enumerate(
                    (
                        (2 * g, 0, 512, 0),
                        (2 * g + 1, 512, 512 + w2, 512 - w2),
                    )
                ):
                    start = first and i == 0
                    stop = last and i == 1
                    vs = v_nat[:, l * VSTRIDE : l * VSTRIDE + 65]
                    nc.tensor.matmul(
                        pso_t[:, oc0:512],
                        lhsT=vs[0:64, :],
                        rhs=p[0:64, pc0:pc1],
                        start=start,
                        stop=stop,
                    )
                    nc.tensor.matmul(
                        pso_t[:, 512 + oc0 : 1024],
                        lhsT=vs[64:128, :],
                        rhs=p[64:128, pc0:pc1],
                        start=start,
                        stop=stop,
                    )

            def emit_O(c, pso_t):
                # single-PSUM-input copy (DVE has one PSUM read port); the
                # host adds the two key-half partials
                o = opool.tile([H + 1, 1024], dt_f16, tag="o")
                nc.vector.tensor_scalar_mul(o, pso_t, 2.0**-6)
                nc.sync.dma_start(out_d.ap()[:, c * 1024 : (c + 1) * 1024], o)

            def proj_filler(c):
                # emitted after the diag PV of chunk c; fills PE idle slots
                if c == 0:
                    kv_proj(1)
                    q_proj(2)
                elif c == 1:
                    q_proj(3)
                elif c == 2:
                    q_proj(4)
                    kv_proj(2)
                elif c == 3:
                    q_proj(5)
                elif c == 4:
                    q_proj(6)
                    kv_proj(3)
                elif c == 5:
                    q_proj(7)

            # ---- software-pipelined main loop: scores run two pairs
            # ahead of exp; PV trails exp by one pair. The DIAGONAL pair
            # goes FIRST in each chunk so the masked PV (the only one
            # gated on the vector engine) is off the chunk-tail chain ----
            all_pairs = [
                (c, g)
                for c in range(NCHUNK)
                for g in ([c] + list(range(c)))
            ]
            ps_map = {}
            sptr = 0

            def pump_S(n):
                nonlocal sptr
                for _ in range(n):
                    if sptr < len(all_pairs):
                        cc, gg = all_pairs[sptr]
                        ps_map[(cc, gg)] = emit_S(cc, gg)
                        sptr += 1

            kv_proj(0)
            q_proj(0)
            q_proj(1)
            pump_S(2)
            pso_t = None
            for i, (c, g) in enumerate(all_pairs):
                first = g == c  # diag pair leads the chunk
                last = (g == c - 1) or (c == 0)
                if first:
                    pso_t = pso.tile([H + 1, 1024], dt_f32, tag="pso")
                p = emit_E(c, g, ps_map.pop((c, g)))
                pump_S(1)
                emit_V(c, g, p, pso_t, first, last)
                if first:
                    proj_filler(c)
                if last:
                    emit_O(c, pso_t)

    nc.compile()
    return nc


def _perm(rho):
    """Rotated-order permutation: rotated position i holds original token
    perm[i]. Involutive (half swap within each 256-block)."""
    i = np.arange(T)
    return (i // 256) * 256 + ((i % 256) + 128 * rho) % 256


def _make_in_maps(x, Wq, bq, Wk, bk, Wv, bv):
    # [Wq|Wq] per e-strip: the partition-64:128 copy of Q feeds the
    # row-tiled score matmuls.
    wq4 = Wq.reshape(4, 128, 64)
    wq2_pack = np.ascontiguousarray(
        np.concatenate([wq4, wq4], axis=2).transpose(1, 0, 2).reshape(128, 512)
    ).astype(bf16)
    wkv_pack = np.ascontiguousarray(
        np.concatenate([Wk.reshape(4, 128, 64), Wv.reshape(4, 128, 64)], axis=2)
        .transpose(1, 0, 2)
        .reshape(128, 512)
    ).astype(bf16)
    bias_q2 = np.ascontiguousarray(
        np.concatenate([bq, bq])[:, None]
    ).astype(np.float32)
    bias_kv = np.ascontiguousarray(np.concatenate([bk, bv])[:, None]).astype(
        np.float32
    )

    kk = np.arange(128)[:, None]
    in_maps = []
    for b in range(B):
        xt_b = np.ascontiguousarray(x[b].T).astype(bf16).reshape(4, 128, T)
        for rho in range(2):
            perm = _perm(rho)
            xt_rot = xt_b[:, :, perm]  # rotated token order
            xt_in = np.ascontiguousarray(
                xt_rot.reshape(4, 128, 4, T // 4).transpose(2, 1, 0, 3)
            )
            # masks: columns in rotated order; v = original within-chunk
            # offset of rotated column j (chunk-independent). m1 is zero
            # on query cols 0:256 for both cores -> only cols 256:512 kept.
            v = perm[:CHUNK]
            m0 = (kk - v[None, :] <= -128 * rho).astype(bf16)
            m1 = (kk - v[None, :] <= -256 - 128 * rho).astype(bf16)
            masks_np = np.ascontiguousarray(
                np.concatenate([m0, m1[:, 256:512]], axis=1)
            )
            in_maps.append(
                {
                    "xt": xt_in,
                    "wq2": wq2_pack,
                    "wkv": wkv_pack,
                    "bias_q2": bias_q2,
                    "bias_kv": bias_kv,
                    "masks": masks_np,
                }
            )
    return in_maps


def _combine(results):
    out = np.empty((B, T, H), np.float32)
    p1 = _perm(1)
    for b in range(B):
        # fold the two key-half partials: [65, 8, 2, 512] -> [65, 4096]
        a0 = (
            results[2 * b]["out"]
            .astype(np.float64)
            .reshape(H + 1, NCHUNK, 2, CHUNK)
            .sum(axis=2)
            .reshape(H + 1, T)
        )
        a1 = (
            results[2 * b + 1]["out"]
            .astype(np.float64)
            .reshape(H + 1, NCHUNK, 2, CHUNK)
            .sum(axis=2)
            .reshape(H + 1, T)
        )
        a1 = a1[:, p1]  # un-rotate core-1 columns (involutive perm)
        num = a0[:H] + a1[:H]
        den = a0[H] + a1[H]
        out[b] = (num / den).T.astype(np.float32)
    return out


def _run(trace=False, **inputs):
    from concourse import bass_utils

    nc = _build()
    in_maps = _make_in_maps(
        np.asarray(inputs["x"], np.float32),
        np.asarray(inputs["Wq"], np.float32),
        np.asarray(inputs["bq"], np.float32),
        np.asarray(inputs["Wk"], np.float32),
        np.asarray(inputs["bk"], np.float32),
        np.asarray(inputs["Wv"], np.float32),
        np.asarray(inputs["bv"], np.float32),
    )
    res = bass_utils.run_bass_kernel_spmd(
        nc, in_maps, list(range(NCORES)), trace=trace
    )
    return _combine(res.results), res.exec_time_ns


def kernel(**inputs):
    out, _ = _run(trace=False, **inputs)
    return out


# revision 19
# speedup vs baseline: 1.1464x; 1.0114x over previous
"""Trainium2 Bass kernel: single-head causal attention.

B=4, T=4096, E=512, H=64, fp32 in/out.

Sharding: 2 cores per batch sample, split by keys (even/odd 128-strips via
a per-256-block half rotation baked in on the host). Each core computes a
partial softmax (numerator + denominator via a ones-column in V) over its
half of the keys for all 4096 queries; the host combines
out = (num0+num1)/(den0+den1).

v2 device kernel (per core):
  - Q projection uses [Wq|Wq] stationary so PSUM rows 0:64 and 64:128 both
    hold Q — the partition-64:128 copy feeds row-tiled score matmuls.
  - K^T is duplicated to partitions 64:128 of a side tile by a small
    SBUF->SBUF DMA on the gpsimd (SWDGE) queue.
  - Scores (contraction H=64) run as two concurrent row-tiled matmuls
    (tile_position (0,0) and (64,0)) per strip pair -> ~2x score rate.
  - PV is split into two 64-row matmuls (key halves) accumulating into two
    PSUM banks, combined during the DVE evacuation add. This keeps the
    whole attention stream in the 64-row PE tiling mode (no mode-switch
    drains between scores and PV).
  - V^T -> V natural transpose moved off the PE to the DMA xbar.
  - Diagonal trim: the 2nd diagonal strip is fully masked for query cols
    0:256 on both cores, so only 768 of 1024 columns are computed.
  - exp on the scalar engine with fused 1/sqrt(H) scale (no max subtract).
  - Warm-up matmuls at kernel start keep the PE HAM clock at 2.4 GHz.
  - Input x DMA is split across both HWDGE queues (scalar queue carries
    quarter 0) so the first projection can start early.
"""

import functools

import numpy as np
import ml_dtypes

B, T, E, H = 4, 4096, 512, 64
NCORES = 8
NCHUNK = 8  # 512-query chunks per sample
CHUNK = T // NCHUNK  # 512
NSTRIP = 16  # local 128-key strips per core (half of T/128)
VSTRIDE = 80  # per-strip stride in the packed V tile
NWARM = 26  # PE warm-up matmuls
PACKED_FROM = 2  # chunks >= this use row-packed scores (kdup ready by then)
# strip pairs whose exp runs on the DVE via the Schraudolph bit-trick
# (softmax normalization cancels most of the ~3% element error; verified
# 2.7e-3 worst-element impact on the real data)
SCH_OFFLOAD = set()  # empty: DVE serialization offset the scalar relief
SCH_MUL = 12102203.16 / 8.0  # log2(e)*2^23 * softmax scale (1/sqrt(64))
SCH_ADD = float(127 * 2**23 - 366500)  # bias calibrated vs np.exp

bf16 = ml_dtypes.bfloat16


@functools.lru_cache(maxsize=1)
def _build():
    import concourse.mybir as mybir
    from concourse import bacc
    import concourse.tile as tile
    from concourse.masks import make_identity

    dt_bf = mybir.dt.bfloat16
    dt_f32 = mybir.dt.float32

    nc = bacc.Bacc("TRN2", target_bir_lowering=False, num_devices=NCORES)

    # x^T, rotated, (quarter, e-strip)-blocked: [4, 128, 4, 1024]
    xt = nc.dram_tensor("xt", [4, 128, 4, T // 4], dt_bf, kind="ExternalInput")
    wq2 = nc.dram_tensor("wq2", [128, 4 * 128], dt_bf, kind="ExternalInput")
    wkv = nc.dram_tensor("wkv", [128, 4 * 128], dt_bf, kind="ExternalInput")
    bias_q2 = nc.dram_tensor("bias_q2", [128, 1], dt_f32, kind="ExternalInput")
    bias_kv = nc.dram_tensor("bias_kv", [128, 1], dt_f32, kind="ExternalInput")
    masks = nc.dram_tensor("masks", [128, 768], dt_bf, kind="ExternalInput")
    # per chunk: [key-half-0 partial | key-half-1 partial], host adds them.
    # fp16 with a 2^-6 scale folded in (cancels in the host's num/den).
    dt_f16 = mybir.dt.float16
    out_d = nc.dram_tensor("out", [H + 1, 2 * T], dt_f16, kind="ExternalOutput")

    scale = 1.0 / float(np.sqrt(H))

    with tile.TileContext(nc) as tc:
        with (
            tc.tile_pool(name="const", bufs=1) as cpool,
            tc.tile_pool(name="xt_pool", bufs=1) as xpool,
            tc.tile_pool(name="q_pool", bufs=3) as qpool,
            tc.tile_pool(name="kv_pool", bufs=4) as kvpool,
            tc.tile_pool(name="kd_pool", bufs=4) as kdpool,
            tc.tile_pool(name="v_pool", bufs=1) as vpool,
            tc.tile_pool(name="p_pool", bufs=4) as ppool,
            tc.tile_pool(name="o_pool", bufs=2) as opool,
            tc.tile_pool(name="i_pool", bufs=2) as ipool,
            tc.tile_pool(name="ps_proj", bufs=2, space="PSUM") as pspr,
            tc.tile_pool(name="ps_s", bufs=2, space="PSUM") as pss,
            tc.tile_pool(name="ps_o", bufs=1, space="PSUM") as pso,
        ):
            # ---- input DMAs: x quarter 0 strictly first on the sync
            # queue (DMA bandwidth is shared across queues, so the
            # critical first quarter must not compete); tiny weight
            # tensors ride the scalar HWDGE queue ----
            xt_sb = xpool.tile([128, 4 * T], dt_bf)

            def xt_block(qd, es):
                off = (qd * 4 + es) * 1024
                return xt_sb[:, off : off + 1024]

            def xt_dma(eng, qd, half):  # half 0 -> es 0,1 ; 1 -> es 2,3
                eng.dma_start(
                    xt_sb[
                        :, (qd * 4 + 2 * half) * 1024 : (qd * 4 + 2 * half + 2) * 1024
                    ],
                    xt.ap()[qd][:, 2 * half : 2 * half + 2, :].rearrange(
                        "p a t -> p (a t)"
                    ),
                )

            wkv_sb = cpool.tile([128, 512], dt_bf)
            nc.sync.dma_start(wkv_sb, wkv.ap())
            xt_dma(nc.sync, 0, 0)
            xt_dma(nc.sync, 0, 1)
            wq2_sb = cpool.tile([128, 512], dt_bf)
            nc.sync.dma_start(wq2_sb, wq2.ap())
            bkv_sb = cpool.tile([128, 1], dt_f32)
            nc.sync.dma_start(bkv_sb, bias_kv.ap())
            bq2_sb = cpool.tile([128, 1], dt_f32)
            nc.sync.dma_start(bq2_sb, bias_q2.ap())
            masks_sb = cpool.tile([128, 768], dt_bf)
            nc.sync.dma_start(masks_sb, masks.ap())
            xt_dma(nc.sync, 1, 0)
            xt_dma(nc.sync, 1, 1)
            xt_dma(nc.sync, 2, 0)
            xt_dma(nc.sync, 2, 1)
            xt_dma(nc.sync, 3, 0)
            xt_dma(nc.sync, 3, 1)

            # ---- PE warm-up: back-to-back junk matmuls flip the HAM
            # clock gate to 2.4 GHz while the first x DMAs land ----
            zt = cpool.tile([128, 128], dt_bf)
            nc.vector.memset(zt, 0.0)
            ident = cpool.tile([128, 128], dt_bf)
            make_identity(nc, ident)
            ps_w = pspr.tile([128, 512], dt_f32, tag="proj")
            for _ in range(NWARM):
                nc.tensor.matmul(ps_w[:, 0:128], lhsT=zt, rhs=zt, start=True, stop=True)

            # packed V (natural [k,h] layout + ones column for denominator)
            v_nat = vpool.tile([128, NSTRIP * VSTRIDE], dt_bf)
            v3 = v_nat.rearrange("p (s c) -> p s c", c=VSTRIDE)
            nc.vector.memset(v3[:, :, 64:65], 1.0)

            kv_tiles = []
            kd_tiles = []
            q_tiles = []

            def kv_proj(ckv, trs=True):
                ps_kv = pspr.tile([128, 512], dt_f32, tag="proj")
                for es in range(4):
                    # keys: first 128 tokens of each 256-block
                    key_rhs = xt_block(ckv, es).rearrange(
                        "p (a two b) -> p a two b", two=2, b=128
                    )[:, :, 0, :]
                    nc.tensor.matmul(
                        ps_kv,
                        lhsT=wkv_sb[:, es * 128 : (es + 1) * 128],
                        rhs=key_rhs,
                        start=(es == 0),
                        stop=(es == 3),
                    )
                kv_sb = kvpool.tile([128, 512], dt_bf, tag="kv")
                nc.vector.tensor_scalar_add(kv_sb, ps_kv, bkv_sb)
                kv_tiles.append(kv_sb)
                # K^T duplicate at partitions 64:128 for row-tiled scores
                kd = kdpool.tile([128, 512], dt_bf, tag="kd")
                nc.gpsimd.dma_start(kd[64:128, :], kv_sb[0:64, :])
                kd_tiles.append(kd)
                if trs:
                    vtrans(ckv)

            def vtrans(ckv):
                # V^T blocks -> natural V strips. Early kv chunks go via
                # the PE (needed within ~1us, batched to minimize PE
                # tiling-mode switches); late ones via the DMA xbar.
                kv_sb = kv_tiles[ckv]
                for j in range(4):
                    s = 4 * ckv + j
                    if ckv < 2:
                        ps_tr = pspr.tile([128, 128], dt_bf, tag="proj")
                        nc.tensor.transpose(
                            ps_tr, kv_sb[:, j * 128 : (j + 1) * 128], ident
                        )
                        nc.vector.tensor_copy(
                            v_nat[:, s * VSTRIDE : s * VSTRIDE + 64],
                            ps_tr[:, 64:128],
                        )
                    else:
                        nc.sync.dma_start(
                            v_nat[:, s * VSTRIDE : s * VSTRIDE + 64],
                            kv_sb[64:128, j * 128 : (j + 1) * 128],
                            transpose=True,
                        )

            def q_proj(c):
                ps_q = pspr.tile([128, 512], dt_f32, tag="proj")
                for es in range(4):
                    nc.tensor.matmul(
                        ps_q,
                        lhsT=wq2_sb[:, es * 128 : (es + 1) * 128],
                        rhs=xt_block(c // 2, es)[
                            :, (c % 2) * CHUNK : (c % 2) * CHUNK + CHUNK
                        ],
                        start=(es == 0),
                        stop=(es == 3),
                    )
                q_sb = qpool.tile([128, 512], dt_bf, tag="q")
                nc.vector.tensor_scalar_add(q_sb, ps_q, bq2_sb)
                q_tiles.append(q_sb)

            def emit_S(c, g):
                """Scores for strip pair g of chunk c: strip 2g (512 query
                cols) and strip 2g+1 (256 cols if diagonal, else 512)."""
                diag = g == c
                w2 = 256 if diag else 512
                ps = pss.tile([128, 1024], dt_f32, tag="pss")
                q = q_tiles[c]
                l0, l1 = 2 * g, 2 * g + 1
                lt0 = kv_tiles[l0 // 4][0:64, (l0 % 4) * 128 : (l0 % 4 + 1) * 128]
                if c >= PACKED_FROM:
                    # concurrent row-tiled pair: (0,0) and (64,0)
                    lt1 = kd_tiles[l1 // 4][64:128, (l1 % 4) * 128 : (l1 % 4 + 1) * 128]
                    r1 = q[64:128, 512 - w2 : 512]
                else:
                    lt1 = kv_tiles[l1 // 4][0:64, (l1 % 4) * 128 : (l1 % 4 + 1) * 128]
                    r1 = q[0:64, 512 - w2 : 512]
                nc.tensor.matmul(
                    ps[:, 0:512], lhsT=lt0, rhs=q[0:64, :], start=True, stop=True
                )
                nc.tensor.matmul(
                    ps[:, 512 : 512 + w2], lhsT=lt1, rhs=r1, start=True, stop=True
                )
                return ps

            def emit_E(c, g, ps):
                diag = g == c
                w = 768 if diag else 1024
                p = ppool.tile([128, 1024], dt_bf, tag="p")
                if (c, g) in SCH_OFFLOAD:
                    # exp on the DVE: i = int(s*log2(e)*2^23/8 + C), then
                    # reinterpret the int32 bits as fp32 (~3% max rel err,
                    # cancels in the softmax normalization)
                    ib = ipool.tile([128, 1024], mybir.dt.int32, tag="ib")
                    nc.vector.tensor_scalar(
                        ib,
                        ps[:, 0:1024],
                        SCH_MUL,
                        SCH_ADD,
                        mybir.AluOpType.mult,
                        mybir.AluOpType.add,
                    )
                    nc.vector.tensor_copy(p, ib[:, :].bitcast(dt_f32))
                    return p
                nc.scalar.activation(
                    p[:, 0:w],
                    ps[:, 0:w],
                    mybir.ActivationFunctionType.Exp,
                    scale=scale,
                )
                if diag:
                    nc.vector.tensor_mul(p[:, 0:768], p[:, 0:768], masks_sb)
                return p

            def emit_V(c, g, p, pso_t, first, last):
                """PV for strip pair g, split into key halves h0/h1 (two
                concurrent 64-row matmuls into separate PSUM banks).
                first/last flag the chunk's accumulation group bounds."""
                diag = g == c
                w2 = 256 if diag else 512
                for i, (l, pc0, pc1, oc0) in enumerate(
                    (
                        (2 * g, 0, 512, 0),
                        (2 * g + 1, 512, 512 + w2, 512 - w2),
                    )
                ):
                    start = first and i == 0
                    stop = last and i == 1
                    vs = v_nat[:, l * VSTRIDE : l * VSTRIDE + 65]
                    nc.tensor.matmul(
                        pso_t[:, oc0:512],
                        lhsT=vs[0:64, :],
                        rhs=p[0:64, pc0:pc1],
                        start=start,
                        stop=stop,
                    )
                    nc.tensor.matmul(
                        pso_t[:, 512 + oc0 : 1024],
                        lhsT=vs[64:128, :],
                        rhs=p[64:128, pc0:pc1],
                        start=start,
                        stop=stop,
                    )

            def emit_O(c, pso_t):
                # single-PSUM-input copy (DVE has one PSUM read port); the
                # host adds the two key-half partials
                o = opool.tile([H + 1, 1024], dt_f16, tag="o")
                nc.vector.tensor_scalar_mul(o, pso_t, 2.0**-6)
                nc.sync.dma_start(out_d.ap()[:, c * 1024 : (c + 1) * 1024], o)

            def proj_filler(c):
                # emitted after the diag PV of chunk c; fills PE idle slots
                if c == 0:
                    kv_proj(1)
                    q_proj(2)
                elif c == 1:
                    q_proj(3)
                elif c == 2:
                    q_proj(4)
                    kv_proj(2)
                elif c == 3:
                    q_proj(5)
                elif c == 4:
                    q_proj(6)
                    kv_proj(3)
                elif c == 5:
                    q_proj(7)

            # ---- software-pipelined main loop: scores run two pairs
            # ahead of exp; PV trails exp by one pair. The DIAGONAL pair
            # goes FIRST in each chunk so the masked PV (the only one
            # gated on the vector engine) is off the chunk-tail chain ----
            all_pairs = [
                (c, g)
                for c in range(NCHUNK)
                for g in ([c] + list(range(c)))
            ]
            ps_map = {}
            sptr = 0

            def pump_S(n):
                nonlocal sptr
                for _ in range(n):
                    if sptr < len(all_pairs):
                        cc, gg = all_pairs[sptr]
                        ps_map[(cc, gg)] = emit_S(cc, gg)
                        sptr += 1

            q_proj(0)
            kv_proj(0, trs=False)
            pump_S(1)  # S(0,0): first exp gates on only q0 + kv0
            vtrans(0)
            q_proj(1)
            pump_S(1)  # S(1,1)
            pso_t = None
            for i, (c, g) in enumerate(all_pairs):
                first = g == c  # diag pair leads the chunk
                last = (g == c - 1) or (c == 0)
                if first:
                    pso_t = pso.tile([H + 1, 1024], dt_f32, tag="pso")
                p = emit_E(c, g, ps_map.pop((c, g)))
                pump_S(1)
                emit_V(c, g, p, pso_t, first, last)
                if first:
                    proj_filler(c)
                if last:
                    emit_O(c, pso_t)

    nc.compile()
    return nc


def _perm(rho):
    """Rotated-order permutation: rotated position i holds original token
    perm[i]. Involutive (half swap within each 256-block)."""
    i = np.arange(T)
    return (i // 256) * 256 + ((i % 256) + 128 * rho) % 256


def _make_in_maps(x, Wq, bq, Wk, bk, Wv, bv):
    # [Wq|Wq] per e-strip: the partition-64:128 copy of Q feeds the
    # row-tiled score matmuls.
    wq4 = Wq.reshape(4, 128, 64)
    wq2_pack = np.ascontiguousarray(
        np.concatenate([wq4, wq4], axis=2).transpose(1, 0, 2).reshape(128, 512)
    ).astype(bf16)
    wkv_pack = np.ascontiguousarray(
        np.concatenate([Wk.reshape(4, 128, 64), Wv.reshape(4, 128, 64)], axis=2)
        .transpose(1, 0, 2)
        .reshape(128, 512)
    ).astype(bf16)
    bias_q2 = np.ascontiguousarray(
        np.concatenate([bq, bq])[:, None]
    ).astype(np.float32)
    bias_kv = np.ascontiguousarray(np.concatenate([bk, bv])[:, None]).astype(
        np.float32
    )

    kk = np.arange(128)[:, None]
    in_maps = []
    for b in range(B):
        xt_b = np.ascontiguousarray(x[b].T).astype(bf16).reshape(4, 128, T)
        for rho in range(2):
            perm = _perm(rho)
            xt_rot = xt_b[:, :, perm]  # rotated token order
            xt_in = np.ascontiguousarray(
                xt_rot.reshape(4, 128, 4, T // 4).transpose(2, 1, 0, 3)
            )
            # masks: columns in rotated order; v = original within-chunk
            # offset of rotated column j (chunk-independent). m1 is zero
            # on query cols 0:256 for both cores -> only cols 256:512 kept.
            v = perm[:CHUNK]
            m0 = (kk - v[None, :] <= -128 * rho).astype(bf16)
            m1 = (kk - v[None, :] <= -256 - 128 * rho).astype(bf16)
            masks_np = np.ascontiguousarray(
                np.concatenate([m0, m1[:, 256:512]], axis=1)
            )
            in_maps.append(
                {
                    "xt": xt_in,
                    "wq2": wq2_pack,
                    "wkv": wkv_pack,
                    "bias_q2": bias_q2,
                    "bias_kv": bias_kv,
                    "masks": masks_np,
                }
            )
    return in_maps


def _combine(results):
    out = np.empty((B, T, H), np.float32)
    p1 = _perm(1)
    for b in range(B):
        # fold the two key-half partials: [65, 8, 2, 512] -> [65, 4096]
        a0 = (
            results[2 * b]["out"]
            .astype(np.float64)
            .reshape(H + 1, NCHUNK, 2, CHUNK)
            .sum(axis=2)
            .reshape(H + 1, T)
        )
        a1 = (
            results[2 * b + 1]["out"]
            .astype(np.float64)
            .reshape(H + 1, NCHUNK, 2, CHUNK)
            .sum(axis=2)
            .reshape(H + 1, T)
        )
        a1 = a1[:, p1]  # un-rotate core-1 columns (involutive perm)
        num = a0[:H] + a1[:H]
        den = a0[H] + a1[H]
        out[b] = (num / den).T.astype(np.float32)
    return out


def _run(trace=False, **inputs):
    from concourse import bass_utils

    nc = _build()
    in_maps = _make_in_maps(
        np.asarray(inputs["x"], np.float32),
        np.asarray(inputs["Wq"], np.float32),
        np.asarray(inputs["bq"], np.float32),
        np.asarray(inputs["Wk"], np.float32),
        np.asarray(inputs["bk"], np.float32),
        np.asarray(inputs["Wv"], np.float32),
        np.asarray(inputs["bv"], np.float32),
    )
    res = bass_utils.run_bass_kernel_spmd(
        nc, in_maps, list(range(NCORES)), trace=trace
    )
    return _combine(res.results), res.exec_time_ns


def kernel(**inputs):
    out, _ = _run(trace=False, **inputs)
    return out
